# revision 26
# baseline (speedup 1.0000x reference)
"""Trainium2 Bass kernel for ConditionedSparseAttention — head-sharded v4.

Problem: B=2, T_IN=2048, T_COND=1024 (S=3072), D=1024, H=16, HD=64, W=512.
The window mask depends only on end_inds[b]: every query attends to the same
1024 keys (rows [e-W, e) of each segment).  Attention reduces to a softmax
over a fixed 1024-key set; K/V projections are needed only for those rows.

Sharding: 8 cores = 2 batches x 4 head-quarters (4 heads / 256 dims each).
Each core computes, for its 4 heads: Q^T projection (all 3072 queries),
K^T/V projections (1024 selected keys; NO cross-core duplication), scores^T
[key, q] -> exp (ScalarE, bf16) -> flipped attn@V out[q, hd+1] with a
ones-augmented V column giving the softmax denominator per query on the
PSUM partition axis -> per-partition normalize (VectorE) -> PE transpose
back to [od, q] -> output-projection PARTIAL y_part = Wo[:, od_mine] @ o^T.
The host sums the 4 partial y's per batch and adds the folded output bias.

All matmuls run in bf16 (full PE rate at any free size); PSUM accumulates
in fp32.  Scores are small (|s| < 4), so softmax needs no max subtraction.

Emission is software-pipelined over (qb, head) units: each unit emits its
four score-groups + exp immediately (keeping the ScalarE exp stream — the
co-critical engine — saturated), the PREVIOUS unit's attn@V groups ride one
unit behind, and all remaining PE work (K/V/Q projections, transposes,
output projection) is a deferred-item queue drained between score groups.
"""
import os
import sys
import tempfile

# The libneuronxla compile cache keys on an HLO hash that does NOT cover the
# embedded BIR payload; pin the cache to a fresh per-process dir so the
# compiled NEFF always matches this code.
os.environ["NEURON_COMPILE_CACHE_URL"] = tempfile.mkdtemp(prefix="bass_kernel_cache_")

try:
    import concourse  # noqa: F401
except ImportError:
    sys.path.insert(0, "/opt/trn_rl_repo")

import numpy as np
import ml_dtypes

import concourse.bacc as bacc
import concourse.tile as tile
import concourse.mybir as mybir
from concourse.bass_utils import run_bass_kernel_spmd

# ---- problem constants (hardcoded per harness contract) ----
B, T_IN, T_COND, D, H, HD, W = 2, 2048, 1024, 1024, 16, 64, 512
S = T_IN + T_COND            # 3072
SEL = 2 * W                  # 1024 selected keys
NCH = D // 128               # 8 input-dim chunks
HPC = 4                      # heads per core
ODC = HPC * HD               # 256 o-dims per core (2 chunks of 128)
QB = 512                     # query block
NQB = S // QB                # 6
KT = SEL // 128              # 8 key tiles

F32 = mybir.dt.float32
BF16 = mybir.dt.bfloat16
AF = mybir.ActivationFunctionType
ALU = mybir.AluOpType

_CACHE = {}


def _build():
    if "nc" in _CACHE:
        return _CACHE["nc"]

    nc = bacc.Bacc("TRN2", target_bir_lowering=False, debug=False,
                   enable_asserts=True, num_devices=8)

    x_d = nc.dram_tensor("x", (128, NCH, S), BF16, kind="ExternalInput").ap()
    xkv_d = nc.dram_tensor("xkv", (128, NCH, SEL), BF16, kind="ExternalInput").ap()
    wqkv_d = nc.dram_tensor("wqkv", (128, NCH, 3 * ODC), BF16,
                            kind="ExternalInput").ap()
    wo_d = nc.dram_tensor("wo", (128, 2, D), BF16, kind="ExternalInput").ap()
    bq_d = nc.dram_tensor("bq", (128, 2), F32, kind="ExternalInput").ap()
    bk_d = nc.dram_tensor("bk", (128, 2), F32, kind="ExternalInput").ap()
    id_d = nc.dram_tensor("ident", (128, 128), BF16, kind="ExternalInput").ap()
    y_d = nc.dram_tensor("y", (128, NCH, S), F32, kind="ExternalOutput").ap()

    with tile.TileContext(nc) as tc:
        with (
            tc.tile_pool(name="const", bufs=1) as cpool,
            tc.tile_pool(name="wts", bufs=1) as wpool,
            tc.tile_pool(name="xin", bufs=1) as xpool,
            tc.tile_pool(name="work", bufs=1) as work,
            tc.tile_pool(name="exps", bufs=3) as epool,
            tc.tile_pool(name="osb", bufs=2) as opool,
            tc.tile_pool(name="recs", bufs=2) as rpool,
            tc.tile_pool(name="ysb", bufs=4) as ypool,
            tc.tile_pool(name="ps", bufs=2, space="PSUM") as ps,       # proj f32 x2 + tp bf16 x1
            tc.tile_pool(name="ps_s", bufs=2, space="PSUM") as ps_s,   # scores [128,2,512] x2
            tc.tile_pool(name="ps_o", bufs=1, space="PSUM") as ps_o,   # attn@V [128,4,65]
        ):
            # ---- tiles ----
            bq_sb = cpool.tile([128, 2], F32, tag="bq")
            bk_sb = cpool.tile([128, 2], F32, tag="bk")
            id_sb = cpool.tile([128, 128], BF16, tag="ident")
            wqkv_sb = wpool.tile([128, NCH, 3 * ODC], BF16, tag="wqkv")
            wo_sb = wpool.tile([128, 2, D], BF16, tag="wo")
            xkv_sb = xpool.tile([128, NCH, SEL], BF16, tag="xkv")
            x_sb = xpool.tile([128, NCH, S], BF16, tag="x")

            # ---- DMAs ordered by first use (DMA engines serialize globally) ----
            nc.scalar.dma_start(wqkv_sb[:], wqkv_d[:])
            nc.sync.dma_start(xkv_sb[:, :, 0:512], xkv_d[:, :, 0:512])
            nc.sync.dma_start(x_sb[:, :, 0:QB], x_d[:, :, 0:QB])
            nc.gpsimd.dma_start(xkv_sb[:, :, 512:1024], xkv_d[:, :, 512:1024])
            nc.scalar.dma_start(bq_sb[:], bq_d[:])
            nc.scalar.dma_start(bk_sb[:], bk_d[:])
            nc.scalar.dma_start(id_sb[:], id_d[:])
            nc.gpsimd.dma_start(wo_sb[:], wo_d[:])
            for qb in range(1, NQB):
                eng = nc.sync if qb % 2 == 1 else nc.gpsimd
                eng.dma_start(x_sb[:, :, qb * QB:(qb + 1) * QB],
                              x_d[:, :, qb * QB:(qb + 1) * QB])

            # ---- PE warm-up during the input-DMA head: dependency-free tiny
            # matmuls ramp the tensor engine to full p-state before real work.
            warm = cpool.tile([128, 128], BF16, tag="warm")
            nc.vector.memset(warm[:], 1.0)
            wps = ps.tile([128, QB], BF16, tag="tp", bufs=1, name="wps")
            for i in range(130):
                nc.tensor.transpose(wps[0:64, 0:64], warm[:, 0:64],
                                    warm[:, 0:64])

            # ---- persistent tensors ----
            q_t = work.tile([128, 2, S], BF16, tag="qt")
            k_t = work.tile([128, 2, SEL], BF16, tag="kt")
            o_all = work.tile([128, 2, S], BF16, tag="oall")
            v_aug = [work.tile([128, HPC, HD + 1], BF16, tag=f"va{kt}",
                               name=f"va{kt}") for kt in range(KT)]

            # ---- deferred-work generators ----
            def kproj(dt, nb):
                psk = ps.tile([128, QB], F32, tag="proj", name=f"psk{dt}{nb}")
                for dc in range(NCH):
                    nc.tensor.matmul(
                        psk[:], wqkv_sb[:, dc, ODC + dt * 128:ODC + (dt + 1) * 128],
                        xkv_sb[:, dc, nb * QB:(nb + 1) * QB],
                        start=(dc == 0), stop=(dc == NCH - 1))
                nc.vector.tensor_scalar(
                    k_t[:, dt, nb * QB:(nb + 1) * QB], psk[:],
                    bk_sb[:, dt:dt + 1], None, ALU.add)

            def qproj(qb, dt):
                psq = ps.tile([128, QB], F32, tag="proj", name=f"psq{dt}_{qb}")
                for dc in range(NCH):
                    nc.tensor.matmul(
                        psq[:], wqkv_sb[:, dc, dt * 128:(dt + 1) * 128],
                        x_sb[:, dc, qb * QB:(qb + 1) * QB],
                        start=(dc == 0), stop=(dc == NCH - 1))
                nc.vector.tensor_scalar(
                    q_t[:, dt, qb * QB:(qb + 1) * QB], psq[:],
                    bq_sb[:, dt:dt + 1], None, ALU.add)

            def vproj_pair(g):
                for kt in (2 * g, 2 * g + 1):
                    psv = ps.tile([128, QB], F32, tag="proj", name=f"psv{kt}")
                    for dc in range(NCH):
                        nc.tensor.matmul(
                            psv[:, 0:ODC], xkv_sb[:, dc, kt * 128:(kt + 1) * 128],
                            wqkv_sb[:, dc, 2 * ODC:3 * ODC],
                            start=(dc == 0), stop=(dc == NCH - 1))
                    nc.vector.tensor_copy(
                        v_aug[kt][:, :, 0:HD],
                        psv[:, 0:ODC].rearrange("p (h d) -> p h d", h=HPC))
                    nc.vector.memset(v_aug[kt][:, :, HD:HD + 1], 1.0)

            def transp(qb, c, o_sb):
                t_ps = ps.tile([128, QB], BF16, tag="tp", bufs=1,
                               name=f"tp{qb}_{c}")
                for qt in range(4):
                    nc.tensor.transpose(
                        t_ps[:, qt * 128:(qt + 1) * 128],
                        o_sb[:, qt, c * 128:(c + 1) * 128], id_sb[:])
                nc.vector.tensor_copy(o_all[:, c, qb * QB:(qb + 1) * QB],
                                      t_ps[:])

            def outproj(qb, dt):
                yp = ps.tile([128, QB], F32, tag="proj", name=f"yp{qb}_{dt}")
                for c in range(2):
                    nc.tensor.matmul(
                        yp[:], wo_sb[:, c, dt * 128:(dt + 1) * 128],
                        o_all[:, c, qb * QB:(qb + 1) * QB],
                        start=(c == 0), stop=(c == 1))
                y_sb = ypool.tile([128, QB], F32, tag="y", name=f"y{qb}_{dt}")
                if qb == NQB - 1 and dt % 2 == 1:
                    # tail: ScalarE is done with exp by now — split the drain
                    nc.scalar.copy(y_sb[:], yp[:])
                else:
                    nc.vector.tensor_copy(y_sb[:], yp[:])
                eng = nc.sync if dt % 2 == 0 else nc.scalar
                eng.dma_start(y_d[:, dt, qb * QB:(qb + 1) * QB], y_sb[:])

            # (cost_ns, fn) deferred queue; ordering respects data deadlines.
            items = [
                (1700, lambda: kproj(0, 1)),
                (1700, lambda: kproj(1, 0)),
                (1700, lambda: kproj(1, 1)),
                (1700, lambda: qproj(0, 1)),
            ]
            items += [(1700, lambda g=g: vproj_pair(g)) for g in range(4)]
            items += [(1700, lambda dt=dt: qproj(1, dt)) for dt in range(2)]

            reserve = []

            def pop_items(budget):
                spent = 0
                while items and spent < budget:
                    c, fn = items.pop(0)
                    fn()
                    spent += c

            # ---- upfront minimal work, then the unit pipeline ----
            qproj(0, 0)
            kproj(0, 0)

            class Unit:
                pass

            prev = None
            units = [(qb, h) for qb in range(NQB) for h in range(HPC)]
            for qb, h in units:
                u = Unit()
                u.qb, u.h = qb, h
                pb = 64 * (h % 2)
                ch = h // 2
                if h == 0:
                    u.o_sb = opool.tile([128, 4, ODC], BF16, tag="osb",
                                        name=f"osb{qb}")
                else:
                    u.o_sb = prev.o_sb
                exp_t = epool.tile([128, KT, QB], BF16, tag="exp",
                                   name=f"exp{qb}_{h}")
                o_ps = ps_o.tile([128, 4, HD + 1], F32, tag="o",
                                 name=f"o{qb}_{h}")

                def attnv(g, exp_t=exp_t, o_ps=o_ps, h=h):
                    for qt in range(4):
                        for j in range(2):
                            kt = 2 * g + j
                            nc.tensor.matmul(
                                o_ps[:, qt, :],
                                exp_t[:, kt, qt * 128:(qt + 1) * 128],
                                v_aug[kt][:, h, :],
                                start=(kt == 0), stop=(kt == KT - 1),
                                skip_group_check=True)

                def norm(qb=qb, h=h, o_ps=o_ps, o_sb=u.o_sb):
                    rec = rpool.tile([128, 4, 1], F32, tag="rec",
                                     name=f"rec{qb}_{h}")
                    nc.vector.reciprocal(rec[:], o_ps[:, :, HD:HD + 1])
                    nc.vector.tensor_tensor(
                        o_sb[:, :, h * HD:(h + 1) * HD], o_ps[:, :, 0:HD],
                        rec.broadcast_to((128, 4, HD)), ALU.mult)

                u.attnv, u.norm = attnv, norm

                for g in range(4):
                    s_ps = ps_s.tile([128, 2, QB], F32, tag="S",
                                     name=f"s{qb}_{h}_{g}")
                    for j in range(2):
                        kt = 2 * g + j
                        nc.tensor.matmul(
                            s_ps[:, j, :],
                            k_t[pb:pb + HD, ch, kt * 128:(kt + 1) * 128],
                            q_t[pb:pb + HD, ch, qb * QB:(qb + 1) * QB],
                            start=True, stop=True, tile_position=(pb, 0))
                    nc.scalar.activation(
                        exp_t[:, 2 * g:2 * g + 2, :], s_ps[:], AF.Exp)
                    if prev is None:
                        pop_items(300)
                    elif g >= 1:
                        pop_items(300)
                        prev.attnv(g - 1)
                pop_items(300)
                if prev is not None:
                    prev.attnv(3)
                    prev.norm()
                    if prev.h == HPC - 1:
                        pqb, posb = prev.qb, prev.o_sb
                        nqb2 = pqb + 2
                        ops = [(400, lambda pqb=pqb, posb=posb:
                                transp(pqb, 0, posb))]
                        if nqb2 < NQB:
                            ops.append((1700, lambda n=nqb2: qproj(n, 0)))
                        ops.append((400, lambda pqb=pqb, posb=posb:
                                    transp(pqb, 1, posb)))
                        if nqb2 < NQB:
                            ops.append((1700, lambda n=nqb2: qproj(n, 1)))
                        ops += [(500, lambda dt=dt, pqb=pqb: outproj(pqb, dt))
                                for dt in range(NCH - 2)]
                        items += ops
                        # reserve the last two outproj tiles to feed the
                        # endgame, where no q-projection filler remains
                        reserve.extend(
                            (500, lambda dt=dt, pqb=pqb: outproj(pqb, dt))
                            for dt in range(NCH - 2, NCH))
                    if (qb, h) == (NQB - 2, 2):
                        items.extend(reserve)
                        reserve.clear()
                    if prev.qb == NQB - 1 and prev.h == 1:
                        # hoist last block's first transpose (heads 0-1 final)
                        items.append((400, lambda posb=u.o_sb:
                                      transp(NQB - 1, 0, posb)))
                prev = u

            # ---- drain: last unit's attn@V + epilogues ----
            for g in range(4):
                prev.attnv(g)
                pop_items(600)
            prev.norm()
            while items:
                items.pop(0)[1]()
            transp(NQB - 1, 1, prev.o_sb)
            for dt in range(NCH):
                outproj(NQB - 1, dt)

    nc.compile()
    _CACHE["nc"] = nc
    return nc


def _to_pko(a2d, dtype=ml_dtypes.bfloat16):
    """(D_in, M) row-major -> [128, D_in//128, M] with d = ko*128 + p."""
    d_in, m = a2d.shape
    return np.ascontiguousarray(
        a2d.reshape(d_in // 128, 128, m).transpose(1, 0, 2)).astype(dtype)


def kernel(x, condition, end_inds, in_proj_w, in_proj_b, out_w, out_b):
    nc = _build()

    x = np.asarray(x, dtype=np.float32)
    condition = np.asarray(condition, dtype=np.float32)
    end_inds = np.asarray(end_inds, dtype=np.int32)
    in_proj_w = np.asarray(in_proj_w, dtype=np.float32)
    in_proj_b = np.asarray(in_proj_b, dtype=np.float32)
    out_w = np.asarray(out_w, dtype=np.float32)
    out_b = np.asarray(out_b, dtype=np.float32)

    ident = np.eye(128, dtype=ml_dtypes.bfloat16)
    wo_full = np.ascontiguousarray(out_w.T)          # (od, ydim)

    in_maps = []
    per_core = []
    for core in range(8):
        b, hq = divmod(core, 4)
        inp = np.concatenate([x[b], condition[b]], axis=0)       # (3072, 1024)
        e = int(end_inds[b])
        sel = np.concatenate([inp[e - W:e], inp[T_IN + e - W:T_IN + e]], axis=0)
        lo = hq * ODC
        wq = 0.125 * in_proj_w[lo:lo + ODC]                      # (256, 1024)
        wk = in_proj_w[D + lo:D + lo + ODC]
        wv = in_proj_w[2 * D + lo:2 * D + lo + ODC]
        bq = np.ascontiguousarray(
            (0.125 * in_proj_b[lo:lo + ODC]).reshape(2, 128).T).astype(np.float32)
        bk = np.ascontiguousarray(
            in_proj_b[D + lo:D + lo + ODC].reshape(2, 128).T).astype(np.float32)
        wqkv = np.concatenate([wq.T, wk.T, wv.T], axis=1)    # (1024, 768)
        in_maps.append({
            "x": _to_pko(np.ascontiguousarray(inp.T)),
            "xkv": _to_pko(np.ascontiguousarray(sel.T)),
            "wqkv": _to_pko(np.ascontiguousarray(wqkv)),
            "wo": _to_pko(np.ascontiguousarray(wo_full[lo:lo + ODC])),
            "bq": bq, "bk": bk, "ident": ident,
        })
        per_core.append((b, hq))

    res = run_bass_kernel_spmd(nc, in_maps, core_ids=list(range(8)))

    out = np.zeros((B, S, D), dtype=np.float32)
    for core in range(8):
        b, hq = per_core[core]
        yv = np.asarray(res.results[core]["y"], dtype=np.float32)  # [128, 8, 3072]
        out[b] += yv.transpose(2, 1, 0).reshape(S, D)
    bo_eff = out_b + out_w @ in_proj_b[2 * D:3 * D]
    out += bo_eff.astype(np.float32)
    return out


# revision 27
# speedup vs baseline: 1.1194x; 1.1194x over previous
"""Trainium2 Bass kernel for ConditionedSparseAttention — head-sharded v4.

Problem: B=2, T_IN=2048, T_COND=1024 (S=3072), D=1024, H=16, HD=64, W=512.
The window mask depends only on end_inds[b]: every query attends to the same
1024 keys (rows [e-W, e) of each segment).  Attention reduces to a softmax
over a fixed 1024-key set; K/V projections are needed only for those rows.

Sharding: 8 cores = 2 batches x 4 head-quarters (4 heads / 256 dims each).
Each core computes, for its 4 heads: Q^T projection (all 3072 queries),
K^T/V projections (1024 selected keys; NO cross-core duplication), scores^T
[key, q] -> exp (ScalarE, bf16) -> flipped attn@V out[q, hd+1] with a
ones-augmented V column giving the softmax denominator per query on the
PSUM partition axis -> per-partition normalize (VectorE) -> PE transpose
back to [od, q] -> output-projection PARTIAL y_part = Wo[:, od_mine] @ o^T.
The host sums the 4 partial y's per batch and adds the folded output bias.

All matmuls run in bf16 (full PE rate at any free size); PSUM accumulates
in fp32.  Scores are small (|s| < 4), so softmax needs no max subtraction.

Emission is software-pipelined over (qb, head) units: each unit emits its
four score-groups + exp immediately (keeping the ScalarE exp stream — the
co-critical engine — saturated), the PREVIOUS unit's attn@V groups ride one
unit behind, and all remaining PE work (K/V/Q projections, transposes,
output projection) is a deferred-item queue drained between score groups.
"""
import os
import sys
import tempfile

# The libneuronxla compile cache keys on an HLO hash that does NOT cover the
# embedded BIR payload; pin the cache to a fresh per-process dir so the
# compiled NEFF always matches this code.
os.environ["NEURON_COMPILE_CACHE_URL"] = tempfile.mkdtemp(prefix="bass_kernel_cache_")

try:
    import concourse  # noqa: F401
except ImportError:
    sys.path.insert(0, "/opt/trn_rl_repo")

import numpy as np
import ml_dtypes

import concourse.bacc as bacc
import concourse.tile as tile
import concourse.mybir as mybir
from concourse.bass_utils import run_bass_kernel_spmd

# ---- problem constants (hardcoded per harness contract) ----
B, T_IN, T_COND, D, H, HD, W = 2, 2048, 1024, 1024, 16, 64, 512
S = T_IN + T_COND            # 3072
SEL = 2 * W                  # 1024 selected keys
NCH = D // 128               # 8 input-dim chunks
HPC = 4                      # heads per core
ODC = HPC * HD               # 256 o-dims per core (2 chunks of 128)
QB = 512                     # query block
NQB = S // QB                # 6
KT = SEL // 128              # 8 key tiles

F32 = mybir.dt.float32
BF16 = mybir.dt.bfloat16
AF = mybir.ActivationFunctionType
ALU = mybir.AluOpType

_CACHE = {}


def _build():
    if "nc" in _CACHE:
        return _CACHE["nc"]

    nc = bacc.Bacc("TRN2", target_bir_lowering=False, debug=False,
                   enable_asserts=True, num_devices=8)

    x_d = nc.dram_tensor("x", (128, NCH, S), BF16, kind="ExternalInput").ap()
    xkv_d = nc.dram_tensor("xkv", (128, NCH, SEL), BF16, kind="ExternalInput").ap()
    wq_d = nc.dram_tensor("wq", (128, NCH, ODC), BF16, kind="ExternalInput").ap()
    wk_d = nc.dram_tensor("wk", (128, NCH, ODC), BF16, kind="ExternalInput").ap()
    wv_d = nc.dram_tensor("wv", (128, NCH, ODC), BF16, kind="ExternalInput").ap()
    wo_d = nc.dram_tensor("wo", (128, 2, D), BF16, kind="ExternalInput").ap()
    bq_d = nc.dram_tensor("bq", (128, 2), F32, kind="ExternalInput").ap()
    bk_d = nc.dram_tensor("bk", (128, 2), F32, kind="ExternalInput").ap()
    id_d = nc.dram_tensor("ident", (128, 128), BF16, kind="ExternalInput").ap()
    y_d = nc.dram_tensor("y", (128, NCH, S), F32, kind="ExternalOutput").ap()

    with tile.TileContext(nc) as tc:
        with (
            tc.tile_pool(name="const", bufs=1) as cpool,
            tc.tile_pool(name="wts", bufs=1) as wpool,
            tc.tile_pool(name="xin", bufs=1) as xpool,
            tc.tile_pool(name="work", bufs=1) as work,
            tc.tile_pool(name="exps", bufs=3) as epool,
            tc.tile_pool(name="osb", bufs=2) as opool,
            tc.tile_pool(name="recs", bufs=2) as rpool,
            tc.tile_pool(name="ysb", bufs=4) as ypool,
            tc.tile_pool(name="ps", bufs=2, space="PSUM") as ps,       # proj f32 x2 + tp bf16 x1
            tc.tile_pool(name="ps_s", bufs=2, space="PSUM") as ps_s,   # scores [128,2,512] x2
            tc.tile_pool(name="ps_o", bufs=1, space="PSUM") as ps_o,   # attn@V [128,4,65]
        ):
            # ---- tiles ----
            bq_sb = cpool.tile([128, 2], F32, tag="bq")
            bk_sb = cpool.tile([128, 2], F32, tag="bk")
            id_sb = cpool.tile([128, 128], BF16, tag="ident")
            wk_sb = wpool.tile([128, NCH, ODC], BF16, tag="wk")
            wq_sb = wpool.tile([128, NCH, ODC], BF16, tag="wq")
            wv_sb = wpool.tile([128, NCH, ODC], BF16, tag="wv")
            wo_sb = wpool.tile([128, 2, D], BF16, tag="wo")
            xkv_sb = xpool.tile([128, NCH, SEL], BF16, tag="xkv")
            x_sb = xpool.tile([128, NCH, S], BF16, tag="x")

            # ---- DMAs ordered by first use (DMA engines serialize globally) ----
            nc.sync.dma_start(wk_sb[:], wk_d[:])
            nc.sync.dma_start(xkv_sb[:, :, 0:512], xkv_d[:, :, 0:512])
            nc.scalar.dma_start(bk_sb[:], bk_d[:])
            nc.scalar.dma_start(wq_sb[:], wq_d[:])
            nc.sync.dma_start(x_sb[:, :, 0:QB], x_d[:, :, 0:QB])
            nc.scalar.dma_start(bq_sb[:], bq_d[:])
            nc.gpsimd.dma_start(xkv_sb[:, :, 512:1024], xkv_d[:, :, 512:1024])
            nc.gpsimd.dma_start(wv_sb[:], wv_d[:])
            nc.scalar.dma_start(id_sb[:], id_d[:])
            nc.gpsimd.dma_start(wo_sb[:], wo_d[:])
            for qb in range(1, NQB):
                eng = nc.sync if qb % 2 == 1 else nc.gpsimd
                eng.dma_start(x_sb[:, :, qb * QB:(qb + 1) * QB],
                              x_d[:, :, qb * QB:(qb + 1) * QB])

            # ---- PE warm-up during the input-DMA head: dependency-free tiny
            # matmuls ramp the tensor engine to full p-state before real work.
            warm = cpool.tile([128, 128], BF16, tag="warm")
            nc.vector.memset(warm[:], 1.0)
            wps = ps.tile([128, QB], BF16, tag="tp", bufs=1, name="wps")
            for i in range(100):
                nc.tensor.transpose(wps[0:64, 0:64], warm[:, 0:64],
                                    warm[:, 0:64])

            # ---- persistent tensors ----
            q_t = work.tile([128, 2, S], BF16, tag="qt")
            k_t = work.tile([128, 2, SEL], BF16, tag="kt")
            o_all = work.tile([128, 2, S], BF16, tag="oall")
            v_aug = [work.tile([128, HPC, HD + 1], BF16, tag=f"va{kt}",
                               name=f"va{kt}") for kt in range(KT)]

            # ---- deferred-work generators ----
            def kproj(dt, nb):
                psk = ps.tile([128, QB], F32, tag="proj", name=f"psk{dt}{nb}")
                for dc in range(NCH):
                    nc.tensor.matmul(
                        psk[:], wk_sb[:, dc, dt * 128:(dt + 1) * 128],
                        xkv_sb[:, dc, nb * QB:(nb + 1) * QB],
                        start=(dc == 0), stop=(dc == NCH - 1))
                nc.vector.tensor_scalar(
                    k_t[:, dt, nb * QB:(nb + 1) * QB], psk[:],
                    bk_sb[:, dt:dt + 1], None, ALU.add)

            def qproj(qb, dt):
                psq = ps.tile([128, QB], F32, tag="proj", name=f"psq{dt}_{qb}")
                for dc in range(NCH):
                    nc.tensor.matmul(
                        psq[:], wq_sb[:, dc, dt * 128:(dt + 1) * 128],
                        x_sb[:, dc, qb * QB:(qb + 1) * QB],
                        start=(dc == 0), stop=(dc == NCH - 1))
                nc.vector.tensor_scalar(
                    q_t[:, dt, qb * QB:(qb + 1) * QB], psq[:],
                    bq_sb[:, dt:dt + 1], None, ALU.add)

            def vproj_pair(g):
                for kt in (2 * g, 2 * g + 1):
                    psv = ps.tile([128, QB], F32, tag="proj", name=f"psv{kt}")
                    for dc in range(NCH):
                        nc.tensor.matmul(
                            psv[:, 0:ODC], xkv_sb[:, dc, kt * 128:(kt + 1) * 128],
                            wv_sb[:, dc, :],
                            start=(dc == 0), stop=(dc == NCH - 1))
                    nc.vector.tensor_copy(
                        v_aug[kt][:, :, 0:HD],
                        psv[:, 0:ODC].rearrange("p (h d) -> p h d", h=HPC))
                    nc.vector.memset(v_aug[kt][:, :, HD:HD + 1], 1.0)

            def transp(qb, c, o_sb):
                t_ps = ps.tile([128, QB], BF16, tag="tp", bufs=1,
                               name=f"tp{qb}_{c}")
                for qt in range(4):
                    nc.tensor.transpose(
                        t_ps[:, qt * 128:(qt + 1) * 128],
                        o_sb[:, qt, c * 128:(c + 1) * 128], id_sb[:])
                nc.vector.tensor_copy(o_all[:, c, qb * QB:(qb + 1) * QB],
                                      t_ps[:])

            def outproj(qb, dt):
                yp = ps.tile([128, QB], F32, tag="proj", name=f"yp{qb}_{dt}")
                for c in range(2):
                    nc.tensor.matmul(
                        yp[:], wo_sb[:, c, dt * 128:(dt + 1) * 128],
                        o_all[:, c, qb * QB:(qb + 1) * QB],
                        start=(c == 0), stop=(c == 1))
                y_sb = ypool.tile([128, QB], F32, tag="y", name=f"y{qb}_{dt}")
                if qb == NQB - 1 and dt % 2 == 1:
                    # tail: ScalarE is done with exp by now — split the drain
                    nc.scalar.copy(y_sb[:], yp[:])
                else:
                    nc.vector.tensor_copy(y_sb[:], yp[:])
                eng = nc.sync if dt % 2 == 0 else nc.gpsimd
                eng.dma_start(y_d[:, dt, qb * QB:(qb + 1) * QB], y_sb[:])

            # (cost_ns, fn) deferred queue; ordering respects data deadlines.
            items = [
                (1700, lambda: kproj(0, 1)),
                (1700, lambda: kproj(1, 0)),
                (1700, lambda: kproj(1, 1)),
                (1700, lambda: qproj(0, 1)),
            ]
            items += [(1700, lambda g=g: vproj_pair(g)) for g in range(4)]
            items += [(1700, lambda dt=dt: qproj(1, dt)) for dt in range(2)]

            reserve = []

            def pop_items(budget):
                spent = 0
                while items and spent < budget:
                    c, fn = items.pop(0)
                    fn()
                    spent += c

            # ---- upfront minimal work, then the unit pipeline ----
            kproj(0, 0)
            qproj(0, 0)

            class Unit:
                pass

            prev = None
            units = [(qb, h) for qb in range(NQB) for h in range(HPC)]
            for qb, h in units:
                u = Unit()
                u.qb, u.h = qb, h
                pb = 64 * (h % 2)
                ch = h // 2
                if h == 0:
                    u.o_sb = opool.tile([128, 4, ODC], BF16, tag="osb",
                                        name=f"osb{qb}")
                else:
                    u.o_sb = prev.o_sb
                exp_t = epool.tile([128, KT, QB], BF16, tag="exp",
                                   name=f"exp{qb}_{h}")
                o_ps = ps_o.tile([128, 4, HD + 1], F32, tag="o",
                                 name=f"o{qb}_{h}")

                def attnv(g, exp_t=exp_t, o_ps=o_ps, h=h):
                    for qt in range(4):
                        for j in range(2):
                            kt = 2 * g + j
                            nc.tensor.matmul(
                                o_ps[:, qt, :],
                                exp_t[:, kt, qt * 128:(qt + 1) * 128],
                                v_aug[kt][:, h, :],
                                start=(kt == 0), stop=(kt == KT - 1),
                                skip_group_check=True)

                def norm(qb=qb, h=h, o_ps=o_ps, o_sb=u.o_sb):
                    rec = rpool.tile([128, 4, 1], F32, tag="rec",
                                     name=f"rec{qb}_{h}")
                    nc.vector.reciprocal(rec[:], o_ps[:, :, HD:HD + 1])
                    nc.vector.tensor_tensor(
                        o_sb[:, :, h * HD:(h + 1) * HD], o_ps[:, :, 0:HD],
                        rec.broadcast_to((128, 4, HD)), ALU.mult)

                u.attnv, u.norm = attnv, norm

                for g in range(4):
                    s_ps = ps_s.tile([128, 2, QB], F32, tag="S",
                                     name=f"s{qb}_{h}_{g}")
                    for j in range(2):
                        kt = 2 * g + j
                        nc.tensor.matmul(
                            s_ps[:, j, :],
                            k_t[pb:pb + HD, ch, kt * 128:(kt + 1) * 128],
                            q_t[pb:pb + HD, ch, qb * QB:(qb + 1) * QB],
                            start=True, stop=True, tile_position=(pb, 0))
                    nc.scalar.activation(
                        exp_t[:, 2 * g:2 * g + 2, :], s_ps[:], AF.Exp)
                    if prev is None:
                        pop_items(300)
                    elif g >= 1:
                        pop_items(300)
                        prev.attnv(g - 1)
                pop_items(300)
                if prev is not None:
                    prev.attnv(3)
                    prev.norm()
                    if prev.h == HPC - 1:
                        pqb, posb = prev.qb, prev.o_sb
                        nqb2 = pqb + 2
                        ops = [(400, lambda pqb=pqb, posb=posb:
                                transp(pqb, 0, posb))]
                        if nqb2 < NQB:
                            ops.append((1700, lambda n=nqb2: qproj(n, 0)))
                        ops.append((400, lambda pqb=pqb, posb=posb:
                                    transp(pqb, 1, posb)))
                        if nqb2 < NQB:
                            ops.append((1700, lambda n=nqb2: qproj(n, 1)))
                        ops += [(500, lambda dt=dt, pqb=pqb: outproj(pqb, dt))
                                for dt in range(NCH - 2)]
                        items += ops
                        # reserve the last two outproj tiles to feed the
                        # endgame, where no q-projection filler remains
                        reserve.extend(
                            (500, lambda dt=dt, pqb=pqb: outproj(pqb, dt))
                            for dt in range(NCH - 2, NCH))
                    if (qb, h) == (NQB - 2, 2):
                        items.extend(reserve)
                        reserve.clear()
                    if prev.qb == NQB - 1 and prev.h == 1:
                        # hoist last block's first transpose (heads 0-1 final)
                        items.append((400, lambda posb=u.o_sb:
                                      transp(NQB - 1, 0, posb)))
                prev = u

            # ---- drain: last unit's attn@V + epilogues ----
            for g in range(4):
                prev.attnv(g)
                pop_items(600)
            prev.norm()
            while items:
                items.pop(0)[1]()
            transp(NQB - 1, 1, prev.o_sb)
            for dt in range(NCH):
                outproj(NQB - 1, dt)

    nc.compile()
    _CACHE["nc"] = nc
    return nc


def _to_pko(a2d, dtype=ml_dtypes.bfloat16):
    """(D_in, M) row-major -> [128, D_in//128, M] with d = ko*128 + p."""
    d_in, m = a2d.shape
    return np.ascontiguousarray(
        a2d.reshape(d_in // 128, 128, m).transpose(1, 0, 2)).astype(dtype)


def kernel(x, condition, end_inds, in_proj_w, in_proj_b, out_w, out_b):
    nc = _build()

    x = np.asarray(x, dtype=np.float32)
    condition = np.asarray(condition, dtype=np.float32)
    end_inds = np.asarray(end_inds, dtype=np.int32)
    in_proj_w = np.asarray(in_proj_w, dtype=np.float32)
    in_proj_b = np.asarray(in_proj_b, dtype=np.float32)
    out_w = np.asarray(out_w, dtype=np.float32)
    out_b = np.asarray(out_b, dtype=np.float32)

    ident = np.eye(128, dtype=ml_dtypes.bfloat16)
    wo_full = np.ascontiguousarray(out_w.T)          # (od, ydim)

    in_maps = []
    per_core = []
    for core in range(8):
        b, hq = divmod(core, 4)
        inp = np.concatenate([x[b], condition[b]], axis=0)       # (3072, 1024)
        e = int(end_inds[b])
        sel = np.concatenate([inp[e - W:e], inp[T_IN + e - W:T_IN + e]], axis=0)
        lo = hq * ODC
        wq = 0.125 * in_proj_w[lo:lo + ODC]                      # (256, 1024)
        wk = in_proj_w[D + lo:D + lo + ODC]
        wv = in_proj_w[2 * D + lo:2 * D + lo + ODC]
        bq = np.ascontiguousarray(
            (0.125 * in_proj_b[lo:lo + ODC]).reshape(2, 128).T).astype(np.float32)
        bk = np.ascontiguousarray(
            in_proj_b[D + lo:D + lo + ODC].reshape(2, 128).T).astype(np.float32)
        in_maps.append({
            "x": _to_pko(np.ascontiguousarray(inp.T)),
            "xkv": _to_pko(np.ascontiguousarray(sel.T)),
            "wq": _to_pko(np.ascontiguousarray(wq.T)),
            "wk": _to_pko(np.ascontiguousarray(wk.T)),
            "wv": _to_pko(np.ascontiguousarray(wv.T)),
            "wo": _to_pko(np.ascontiguousarray(wo_full[lo:lo + ODC])),
            "bq": bq, "bk": bk, "ident": ident,
        })
        per_core.append((b, hq))

    res = run_bass_kernel_spmd(nc, in_maps, core_ids=list(range(8)))

    out = np.zeros((B, S, D), dtype=np.float32)
    for core in range(8):
        b, hq = per_core[core]
        yv = np.asarray(res.results[core]["y"], dtype=np.float32)  # [128, 8, 3072]
        out[b] += yv.transpose(2, 1, 0).reshape(S, D)
    bo_eff = out_b + out_w @ in_proj_b[2 * D:3 * D]
    out += bo_eff.astype(np.float32)
    return out


# revision 28
# speedup vs baseline: 1.1356x; 1.0144x over previous
"""Trainium2 Bass kernel for ConditionedSparseAttention — head-sharded v4.

Problem: B=2, T_IN=2048, T_COND=1024 (S=3072), D=1024, H=16, HD=64, W=512.
The window mask depends only on end_inds[b]: every query attends to the same
1024 keys (rows [e-W, e) of each segment).  Attention reduces to a softmax
over a fixed 1024-key set; K/V projections are needed only for those rows.

Sharding: 8 cores = 2 batches x 4 head-quarters (4 heads / 256 dims each).
Each core computes, for its 4 heads: Q^T projection (all 3072 queries),
K^T/V projections (1024 selected keys; NO cross-core duplication), scores^T
[key, q] -> exp (ScalarE, bf16) -> flipped attn@V out[q, hd+1] with a
ones-augmented V column giving the softmax denominator per query on the
PSUM partition axis -> per-partition normalize (VectorE) -> PE transpose
back to [od, q] -> output-projection PARTIAL y_part = Wo[:, od_mine] @ o^T.
The host sums the 4 partial y's per batch and adds the folded output bias.

All matmuls run in bf16 (full PE rate at any free size); PSUM accumulates
in fp32.  Scores are small (|s| < 4), so softmax needs no max subtraction.

Emission is software-pipelined over (qb, head) units: each unit emits its
four score-groups + exp immediately (keeping the ScalarE exp stream — the
co-critical engine — saturated), the PREVIOUS unit's attn@V groups ride one
unit behind, and all remaining PE work (K/V/Q projections, transposes,
output projection) is a deferred-item queue drained between score groups.
"""
import os
import sys
import tempfile

# The libneuronxla compile cache keys on an HLO hash that does NOT cover the
# embedded BIR payload; pin the cache to a fresh per-process dir so the
# compiled NEFF always matches this code.
os.environ["NEURON_COMPILE_CACHE_URL"] = tempfile.mkdtemp(prefix="bass_kernel_cache_")

try:
    import concourse  # noqa: F401
except ImportError:
    sys.path.insert(0, "/opt/trn_rl_repo")

import numpy as np
import ml_dtypes

import concourse.bacc as bacc
import concourse.tile as tile
import concourse.mybir as mybir
from concourse.bass_utils import run_bass_kernel_spmd

# ---- problem constants (hardcoded per harness contract) ----
B, T_IN, T_COND, D, H, HD, W = 2, 2048, 1024, 1024, 16, 64, 512
S = T_IN + T_COND            # 3072
SEL = 2 * W                  # 1024 selected keys
NCH = D // 128               # 8 input-dim chunks
HPC = 4                      # heads per core
ODC = HPC * HD               # 256 o-dims per core (2 chunks of 128)
QB = 512                     # query block
NQB = S // QB                # 6
KT = SEL // 128              # 8 key tiles

F32 = mybir.dt.float32
BF16 = mybir.dt.bfloat16
AF = mybir.ActivationFunctionType
ALU = mybir.AluOpType

_CACHE = {}


def _build():
    if "nc" in _CACHE:
        return _CACHE["nc"]

    nc = bacc.Bacc("TRN2", target_bir_lowering=False, debug=False,
                   enable_asserts=True, num_devices=8)

    x_d = nc.dram_tensor("x", (128, NCH, S), BF16, kind="ExternalInput").ap()
    xkv_d = nc.dram_tensor("xkv", (128, NCH, SEL), BF16, kind="ExternalInput").ap()
    wq_d = nc.dram_tensor("wq", (128, NCH, ODC), BF16, kind="ExternalInput").ap()
    wk_d = nc.dram_tensor("wk", (128, NCH, ODC), BF16, kind="ExternalInput").ap()
    wv_d = nc.dram_tensor("wv", (128, NCH, ODC), BF16, kind="ExternalInput").ap()
    wo_d = nc.dram_tensor("wo", (128, 2, D), BF16, kind="ExternalInput").ap()
    bq_d = nc.dram_tensor("bq", (128, 2), F32, kind="ExternalInput").ap()
    bk_d = nc.dram_tensor("bk", (128, 2), F32, kind="ExternalInput").ap()
    id_d = nc.dram_tensor("ident", (128, 128), BF16, kind="ExternalInput").ap()
    y_d = nc.dram_tensor("y", (128, NCH, S), BF16, kind="ExternalOutput").ap()

    with tile.TileContext(nc) as tc:
        with (
            tc.tile_pool(name="const", bufs=1) as cpool,
            tc.tile_pool(name="wts", bufs=1) as wpool,
            tc.tile_pool(name="xin", bufs=1) as xpool,
            tc.tile_pool(name="work", bufs=1) as work,
            tc.tile_pool(name="exps", bufs=3) as epool,
            tc.tile_pool(name="osb", bufs=2) as opool,
            tc.tile_pool(name="recs", bufs=2) as rpool,
            tc.tile_pool(name="ysb", bufs=4) as ypool,
            tc.tile_pool(name="ps", bufs=2, space="PSUM") as ps,       # proj f32 x2 + tp bf16 x1
            tc.tile_pool(name="ps_s", bufs=2, space="PSUM") as ps_s,   # scores [128,2,512] x2
            tc.tile_pool(name="ps_o", bufs=1, space="PSUM") as ps_o,   # attn@V [128,4,65]
        ):
            # ---- tiles ----
            bq_sb = cpool.tile([128, 2], F32, tag="bq")
            bk_sb = cpool.tile([128, 2], F32, tag="bk")
            id_sb = cpool.tile([128, 128], BF16, tag="ident")
            wk_sb = wpool.tile([128, NCH, ODC], BF16, tag="wk")
            wq_sb = wpool.tile([128, NCH, ODC], BF16, tag="wq")
            wv_sb = wpool.tile([128, NCH, ODC], BF16, tag="wv")
            wo_sb = wpool.tile([128, 2, D], BF16, tag="wo")
            xkv_sb = xpool.tile([128, NCH, SEL], BF16, tag="xkv")
            x_sb = xpool.tile([128, NCH, S], BF16, tag="x")

            # ---- DMAs ordered by first use (DMA engines serialize globally) ----
            nc.sync.dma_start(wk_sb[:], wk_d[:])
            nc.sync.dma_start(xkv_sb[:, :, 0:512], xkv_d[:, :, 0:512])
            nc.scalar.dma_start(bk_sb[:], bk_d[:])
            nc.scalar.dma_start(wq_sb[:], wq_d[:])
            nc.sync.dma_start(x_sb[:, :, 0:QB], x_d[:, :, 0:QB])
            nc.scalar.dma_start(bq_sb[:], bq_d[:])
            nc.gpsimd.dma_start(xkv_sb[:, :, 512:1024], xkv_d[:, :, 512:1024])
            nc.gpsimd.dma_start(wv_sb[:], wv_d[:])
            nc.scalar.dma_start(id_sb[:], id_d[:])
            nc.gpsimd.dma_start(wo_sb[:], wo_d[:])
            for qb in range(1, NQB):
                eng = nc.sync if qb % 2 == 1 else nc.gpsimd
                eng.dma_start(x_sb[:, :, qb * QB:(qb + 1) * QB],
                              x_d[:, :, qb * QB:(qb + 1) * QB])

            # ---- PE warm-up during the input-DMA head: dependency-free tiny
            # matmuls ramp the tensor engine to full p-state before real work.
            warm = cpool.tile([128, 128], BF16, tag="warm")
            nc.vector.memset(warm[:], 1.0)
            wps = ps.tile([128, QB], BF16, tag="tp", bufs=1, name="wps")
            for i in range(190):
                nc.tensor.transpose(wps[0:64, 0:64], warm[:, 0:64],
                                    warm[:, 0:64])

            # ---- persistent tensors ----
            q_t = work.tile([128, 2, S], BF16, tag="qt")
            k_t = work.tile([128, 2, SEL], BF16, tag="kt")
            o_all = work.tile([128, 2, S], BF16, tag="oall")
            v_aug = [work.tile([128, HPC, HD + 1], BF16, tag=f"va{kt}",
                               name=f"va{kt}") for kt in range(KT)]

            # ---- deferred-work generators ----
            def kproj(dt, nb):
                psk = ps.tile([128, QB], F32, tag="proj", name=f"psk{dt}{nb}")
                for dc in range(NCH):
                    nc.tensor.matmul(
                        psk[:], wk_sb[:, dc, dt * 128:(dt + 1) * 128],
                        xkv_sb[:, dc, nb * QB:(nb + 1) * QB],
                        start=(dc == 0), stop=(dc == NCH - 1))
                nc.vector.tensor_scalar(
                    k_t[:, dt, nb * QB:(nb + 1) * QB], psk[:],
                    bk_sb[:, dt:dt + 1], None, ALU.add)

            def qproj(qb, dt):
                psq = ps.tile([128, QB], F32, tag="proj", name=f"psq{dt}_{qb}")
                for dc in range(NCH):
                    nc.tensor.matmul(
                        psq[:], wq_sb[:, dc, dt * 128:(dt + 1) * 128],
                        x_sb[:, dc, qb * QB:(qb + 1) * QB],
                        start=(dc == 0), stop=(dc == NCH - 1))
                nc.vector.tensor_scalar(
                    q_t[:, dt, qb * QB:(qb + 1) * QB], psq[:],
                    bq_sb[:, dt:dt + 1], None, ALU.add)

            def vproj_pair(g):
                for kt in (2 * g, 2 * g + 1):
                    psv = ps.tile([128, QB], F32, tag="proj", name=f"psv{kt}")
                    for dc in range(NCH):
                        nc.tensor.matmul(
                            psv[:, 0:ODC], xkv_sb[:, dc, kt * 128:(kt + 1) * 128],
                            wv_sb[:, dc, :],
                            start=(dc == 0), stop=(dc == NCH - 1))
                    nc.vector.tensor_copy(
                        v_aug[kt][:, :, 0:HD],
                        psv[:, 0:ODC].rearrange("p (h d) -> p h d", h=HPC))
                    nc.vector.memset(v_aug[kt][:, :, HD:HD + 1], 1.0)

            def transp(qb, c, o_sb):
                t_ps = ps.tile([128, QB], BF16, tag="tp", bufs=1,
                               name=f"tp{qb}_{c}")
                for qt in range(4):
                    nc.tensor.transpose(
                        t_ps[:, qt * 128:(qt + 1) * 128],
                        o_sb[:, qt, c * 128:(c + 1) * 128], id_sb[:])
                nc.vector.tensor_copy(o_all[:, c, qb * QB:(qb + 1) * QB],
                                      t_ps[:])

            def outproj(qb, dt):
                yp = ps.tile([128, QB], F32, tag="proj", name=f"yp{qb}_{dt}")
                for c in range(2):
                    nc.tensor.matmul(
                        yp[:], wo_sb[:, c, dt * 128:(dt + 1) * 128],
                        o_all[:, c, qb * QB:(qb + 1) * QB],
                        start=(c == 0), stop=(c == 1))
                y_sb = ypool.tile([128, QB], BF16, tag="y", name=f"y{qb}_{dt}")
                if qb == NQB - 1 and dt % 2 == 1:
                    # tail: ScalarE is done with exp by now — split the drain
                    nc.scalar.copy(y_sb[:], yp[:])
                else:
                    nc.vector.tensor_copy(y_sb[:], yp[:])
                eng = nc.sync if dt % 2 == 0 else nc.gpsimd
                eng.dma_start(y_d[:, dt, qb * QB:(qb + 1) * QB], y_sb[:])

            # (cost_ns, fn) deferred queue; ordering respects data deadlines.
            items = [
                (1700, lambda: kproj(0, 1)),
                (1700, lambda: kproj(1, 0)),
                (1700, lambda: kproj(1, 1)),
                (1700, lambda: qproj(0, 1)),
            ]
            items += [(1700, lambda g=g: vproj_pair(g)) for g in range(4)]
            items += [(1700, lambda dt=dt: qproj(1, dt)) for dt in range(2)]

            reserve = []

            def pop_items(budget):
                spent = 0
                while items and spent < budget:
                    c, fn = items.pop(0)
                    fn()
                    spent += c

            # ---- upfront minimal work, then the unit pipeline ----
            kproj(0, 0)
            qproj(0, 0)

            class Unit:
                pass

            prev = None
            units = [(qb, h) for qb in range(NQB) for h in range(HPC)]
            for qb, h in units:
                u = Unit()
                u.qb, u.h = qb, h
                pb = 64 * (h % 2)
                ch = h // 2
                if h == 0:
                    u.o_sb = opool.tile([128, 4, ODC], BF16, tag="osb",
                                        name=f"osb{qb}")
                else:
                    u.o_sb = prev.o_sb
                exp_t = epool.tile([128, KT, QB], BF16, tag="exp",
                                   name=f"exp{qb}_{h}")
                o_ps = ps_o.tile([128, 4, HD + 1], F32, tag="o",
                                 name=f"o{qb}_{h}")

                def attnv(g, exp_t=exp_t, o_ps=o_ps, h=h):
                    for qt in range(4):
                        for j in range(2):
                            kt = 2 * g + j
                            nc.tensor.matmul(
                                o_ps[:, qt, :],
                                exp_t[:, kt, qt * 128:(qt + 1) * 128],
                                v_aug[kt][:, h, :],
                                start=(kt == 0), stop=(kt == KT - 1),
                                skip_group_check=True)

                def norm(qb=qb, h=h, o_ps=o_ps, o_sb=u.o_sb):
                    rec = rpool.tile([128, 4, 1], F32, tag="rec",
                                     name=f"rec{qb}_{h}")
                    nc.vector.reciprocal(rec[:], o_ps[:, :, HD:HD + 1])
                    nc.vector.tensor_tensor(
                        o_sb[:, :, h * HD:(h + 1) * HD], o_ps[:, :, 0:HD],
                        rec.broadcast_to((128, 4, HD)), ALU.mult)

                u.attnv, u.norm = attnv, norm

                for g in range(4):
                    s_ps = ps_s.tile([128, 2, QB], F32, tag="S",
                                     name=f"s{qb}_{h}_{g}")
                    for j in range(2):
                        kt = 2 * g + j
                        nc.tensor.matmul(
                            s_ps[:, j, :],
                            k_t[pb:pb + HD, ch, kt * 128:(kt + 1) * 128],
                            q_t[pb:pb + HD, ch, qb * QB:(qb + 1) * QB],
                            start=True, stop=True, tile_position=(pb, 0))
                    nc.scalar.activation(
                        exp_t[:, 2 * g:2 * g + 2, :], s_ps[:], AF.Exp)
                    if prev is None:
                        pop_items(300)
                    elif g >= 1:
                        pop_items(300)
                        prev.attnv(g - 1)
                pop_items(300)
                if prev is not None:
                    prev.attnv(3)
                    prev.norm()
                    if prev.h == HPC - 1:
                        pqb, posb = prev.qb, prev.o_sb
                        nqb2 = pqb + 2
                        ops = [(400, lambda pqb=pqb, posb=posb:
                                transp(pqb, 0, posb))]
                        if nqb2 < NQB:
                            ops.append((1700, lambda n=nqb2: qproj(n, 0)))
                        ops.append((400, lambda pqb=pqb, posb=posb:
                                    transp(pqb, 1, posb)))
                        if nqb2 < NQB:
                            ops.append((1700, lambda n=nqb2: qproj(n, 1)))
                        ops += [(500, lambda dt=dt, pqb=pqb: outproj(pqb, dt))
                                for dt in range(NCH - 2)]
                        items += ops
                        # reserve the last two outproj tiles to feed the
                        # endgame, where no q-projection filler remains
                        reserve.extend(
                            (500, lambda dt=dt, pqb=pqb: outproj(pqb, dt))
                            for dt in range(NCH - 2, NCH))
                    if (qb, h) == (NQB - 2, 2):
                        items.extend(reserve)
                        reserve.clear()
                    if prev.qb == NQB - 1 and prev.h == 1:
                        # hoist last block's first transpose (heads 0-1 final)
                        items.append((400, lambda posb=u.o_sb:
                                      transp(NQB - 1, 0, posb)))
                prev = u

            # ---- drain: last unit's attn@V + epilogues ----
            for g in range(4):
                prev.attnv(g)
                pop_items(600)
            prev.norm()
            while items:
                items.pop(0)[1]()
            transp(NQB - 1, 1, prev.o_sb)
            for dt in range(NCH):
                outproj(NQB - 1, dt)

    nc.compile()
    _CACHE["nc"] = nc
    return nc


def _to_pko(a2d, dtype=ml_dtypes.bfloat16):
    """(D_in, M) row-major -> [128, D_in//128, M] with d = ko*128 + p."""
    d_in, m = a2d.shape
    return np.ascontiguousarray(
        a2d.reshape(d_in // 128, 128, m).transpose(1, 0, 2)).astype(dtype)


def kernel(x, condition, end_inds, in_proj_w, in_proj_b, out_w, out_b):
    nc = _build()

    x = np.asarray(x, dtype=np.float32)
    condition = np.asarray(condition, dtype=np.float32)
    end_inds = np.asarray(end_inds, dtype=np.int32)
    in_proj_w = np.asarray(in_proj_w, dtype=np.float32)
    in_proj_b = np.asarray(in_proj_b, dtype=np.float32)
    out_w = np.asarray(out_w, dtype=np.float32)
    out_b = np.asarray(out_b, dtype=np.float32)

    ident = np.eye(128, dtype=ml_dtypes.bfloat16)
    wo_full = np.ascontiguousarray(out_w.T)          # (od, ydim)

    in_maps = []
    per_core = []
    for core in range(8):
        b, hq = divmod(core, 4)
        inp = np.concatenate([x[b], condition[b]], axis=0)       # (3072, 1024)
        e = int(end_inds[b])
        sel = np.concatenate([inp[e - W:e], inp[T_IN + e - W:T_IN + e]], axis=0)
        lo = hq * ODC
        wq = 0.125 * in_proj_w[lo:lo + ODC]                      # (256, 1024)
        wk = in_proj_w[D + lo:D + lo + ODC]
        wv = in_proj_w[2 * D + lo:2 * D + lo + ODC]
        bq = np.ascontiguousarray(
            (0.125 * in_proj_b[lo:lo + ODC]).reshape(2, 128).T).astype(np.float32)
        bk = np.ascontiguousarray(
            in_proj_b[D + lo:D + lo + ODC].reshape(2, 128).T).astype(np.float32)
        in_maps.append({
            "x": _to_pko(np.ascontiguousarray(inp.T)),
            "xkv": _to_pko(np.ascontiguousarray(sel.T)),
            "wq": _to_pko(np.ascontiguousarray(wq.T)),
            "wk": _to_pko(np.ascontiguousarray(wk.T)),
            "wv": _to_pko(np.ascontiguousarray(wv.T)),
            "wo": _to_pko(np.ascontiguousarray(wo_full[lo:lo + ODC])),
            "bq": bq, "bk": bk, "ident": ident,
        })
        per_core.append((b, hq))

    res = run_bass_kernel_spmd(nc, in_maps, core_ids=list(range(8)))

    out = np.zeros((B, S, D), dtype=np.float32)
    for core in range(8):
        b, hq = per_core[core]
        yv = np.asarray(res.results[core]["y"], dtype=np.float32)  # [128, 8, 3072]
        out[b] += yv.transpose(2, 1, 0).reshape(S, D)
    bo_eff = out_b + out_w @ in_proj_b[2 * D:3 * D]
    out += bo_eff.astype(np.float32)
    return out
